# revision 1
# baseline (speedup 1.0000x reference)
"""Deformable-attention transformer layer — TRN2 Bass kernel (per-core shard).

Each core: 1024 queries x 2 batches (2048 rows); value/weights replicated.
v = b*1024 + qlocal indexes queries in natural shard order.
Gather streams per (b,h): 48 j-slots (j = blk*12 + lp; blk=(row,x); lp=(l,p)),
u-scrambled within each 1024-query j-block: stream position u carries query
v(u) = (u%16)*64 + u//16, making the int16 index wrap DMA-contiguous.
Tables per stack (=batch): [128 = h*16+cpair, 6300] fp32 lanes holding bf16
channel pairs (2p, 2p+1) at pixel px (p = partition).
"""
import numpy as np
from contextlib import ExitStack

import concourse.bass as bass
import concourse.mybir as mybir
import concourse.tile as tile

dt = mybir.dt
alu = mybir.AluOpType
ACTF = mybir.ActivationFunctionType
AX = mybir.AxisListType

B = 2
NQS = 1024
NQT = B * NQS
C = 256
H = 8
L = 3
P = 4
NV = 6300
WS = [80, 40, 20]
HS = [60, 30, 15]
STARTS = [0, 4800, 6000]
NLP = L * P          # 12
NHLP = H * NLP       # 96
NJ = 48
JC = 3               # j-slots per gather chunk
NCHUNK = NJ // JC    # 16
CHL = JC * NQS       # 3072 lanes / chunk
F32 = dt.float32
BF16 = dt.bfloat16
I16 = dt.int16
I32 = dt.int32


def host_consts():
    cc = np.zeros((NHLP, 8), np.float32)
    for h in range(H):
        for l in range(L):
            for p in range(P):
                r = h * NLP + l * P + p
                cc[r] = [WS[l], WS[l] - 1, WS[l] - 2,
                         HS[l], HS[l] - 1, HS[l] - 2,
                         WS[l], STARTS[l]]
    return {"ident": np.eye(128, dtype=np.float32), "ccols": cc}


def build(nc):
    dr = {}

    def din(name, shape, dtype=F32):
        dr[name] = nc.dram_tensor(name, shape, dtype, kind="ExternalInput").ap()

    din("query", (NQT, C)); din("value", (B, NV, C)); din("query_pos", (NQT, C))
    din("ref_pts", (NQT, L * 2))
    din("g1", (1, C)); din("b1", (1, C))
    din("Wo", (C, 192)); din("bo", (1, 192))
    din("Wa", (C, 96)); din("ba", (1, 96))
    din("Wv", (C, C)); din("bv", (1, C))
    din("Wp", (C, C)); din("bp", (1, C))
    din("g2", (1, C)); din("b2", (1, C))
    din("Wf1", (C, 4 * C)); din("bf1", (1, 4 * C))
    din("Wf2", (4 * C, C)); din("bf2", (1, C))
    din("ident", (128, 128)); din("ccols", (NHLP, 8))
    dr["out"] = nc.dram_tensor("out", (NQT, C), F32, kind="ExternalOutput").ap()

    with ExitStack() as ctx:
        tc = ctx.enter_context(tile.TileContext(nc))
        _trace(ctx, tc, nc, dr)
    return dr


def _trace(ctx, tc, nc, dr):
    perm = ctx.enter_context(tc.tile_pool(name="perm", bufs=1))
    dramp = ctx.enter_context(tc.tile_pool(name="dramp", bufs=1, space="DRAM"))
    psp = ctx.enter_context(tc.tile_pool(name="psp", bufs=2, space="PSUM"))
    scr = ctx.enter_context(tc.tile_pool(name="scr", bufs=2))

    # ---- constants ----
    ident_f = perm.tile([128, 128], F32, tag="ident_f")
    nc.sync.dma_start(ident_f[:], dr["ident"])
    ident_b = perm.tile([128, 128], BF16, tag="ident_b")
    nc.scalar.activation(ident_b[:], ident_f[:], ACTF.Copy)
    cc = perm.tile([NHLP, 8], F32, tag="ccols")
    nc.sync.dma_start(cc[:], dr["ccols"])

    def col(k):
        return cc[:, k:k + 1]

    ones_f = perm.tile([128, 1], F32, tag="ones_f")
    nc.vector.memset(ones_f[:], 1.0)

    # weights as lists of <=128-row bf16 slabs
    def load_bf16(name, rows, cols, tag):
        slabs = []
        for i in range(rows // 128):
            t32 = scr.tile([128, cols], F32, tag="w32")
            nc.sync.dma_start(t32[:], dr[name][i * 128:(i + 1) * 128, :])
            tb = perm.tile([128, cols], BF16, tag=f"{tag}{i}")
            nc.scalar.activation(tb[:], t32[:], ACTF.Copy)
            slabs.append(tb)
        return slabs

    Wo_b = load_bf16("Wo", C, 192, "Wo")
    Wa_b = load_bf16("Wa", C, 96, "Wa")
    Wv_b = load_bf16("Wv", C, C, "Wv")
    Wf1_b = load_bf16("Wf1", C, 4 * C, "Wf1")
    Wf2_b = load_bf16("Wf2", 4 * C, C, "Wf2")

    Wp_par = []
    for par in range(2):
        t32 = scr.tile([128, C], F32, tag="w32")
        nc.sync.dma_start(
            t32[:], dr["Wp"].rearrange("(hc two) c -> hc two c", two=2)[:, par:par + 1, :])
        tb = perm.tile([128, C], BF16, tag=f"Wp{par}")
        nc.scalar.activation(tb[:], t32[:], ACTF.Copy)
        Wp_par.append(tb)

    def tcol(row, n=C):
        outc = []
        for hf in range(n // 128):
            t = perm.tile([128, 1], F32, tag=f"tc_{row}{hf}")
            nc.sync.dma_start(t[:], dr[row][0:1, hf * 128:(hf + 1) * 128])
            outc.append(t)
        return outc

    bp_c = tcol("bp"); g2_c = tcol("g2"); b2_c = tcol("b2")
    g1_c = tcol("g1"); b1_c = tcol("b1"); bf2_c = tcol("bf2")
    bf1_c = tcol("bf1", 4 * C)
    bo_c = []
    for xy in range(2):
        t = perm.tile([NHLP, 1], F32, tag=f"bo{xy}")
        nc.sync.dma_start(
            t[:], dr["bo"][0:1, :].rearrange("one (r two) -> one r two", two=2)[:, :, xy:xy + 1])
        bo_c.append(t)
    bv_c = []
    for par in range(2):
        t = perm.tile([128, 1], F32, tag=f"bv{par}")
        nc.sync.dma_start(
            t[:], dr["bv"][0:1, :].rearrange("one (hc two) -> one hc two", two=2)[:, :, par:par + 1])
        bv_c.append(t)
    ba_row = perm.tile([1, 96], F32, tag="ba_row")
    nc.sync.dma_start(ba_row[:], dr["ba"])

    def bcast_row(row_ap, n, tag, pool):
        stage = scr.tile([128, n], F32, tag="bcst")
        for qd in range(4):
            nc.sync.dma_start(stage[32 * qd:32 * qd + 1, :], row_ap)
        outt = pool.tile([128, n], F32, tag=tag)
        nc.vector.stream_shuffle(outt[:], stage[:], [0] * 32)
        return outt

    baT = bcast_row(ba_row[:], 96, "baT", perm)

    # ---- phase 1: queryT/qposT transposes, LN1, qaT ----
    qaT = [perm.tile([128, NQT], BF16, tag=f"qaT{i}") for i in range(2)]
    qnT_d = dramp.tile([128, 2 * NQT], F32, tag="qnT_d")
    qT_d = dramp.tile([128, 2 * NQT], F32, tag="qT_d")

    with tc.tile_pool(name="p1", bufs=2) as p1:
        qT = [p1.tile([128, NQT], F32, tag=f"qT{i}") for i in range(2)]
        pT = [p1.tile([128, NQT], F32, tag=f"pT{i}") for i in range(2)]
        for dst, name in ((qT, "query"), (pT, "query_pos")):
            for t in range(16):
                tl = p1.tile([128, C], F32, tag="ld")
                nc.sync.dma_start(tl[:], dr[name][t * 128:(t + 1) * 128, :])
                for hf in range(2):
                    ps = psp.tile([128, 128], F32, tag="tp")
                    nc.tensor.transpose(ps[:], tl[:, hf * 128:(hf + 1) * 128],
                                        ident_f[:])
                    nc.scalar.activation(dst[hf][:, t * 128:(t + 1) * 128],
                                         ps[:], ACTF.Copy)

        srow = p1.tile([1, NQT], F32, tag="l1s")
        s2row = p1.tile([1, NQT], F32, tag="l1s2")
        for chu in range(NQT // 512):
            sl = slice(chu * 512, (chu + 1) * 512)
            ps = psp.tile([1, 512], F32, tag="ps1")
            ps2 = psp.tile([1, 512], F32, tag="ps2")
            for hf in range(2):
                nc.tensor.matmul(ps[:], ones_f[:], qT[hf][:, sl],
                                 start=(hf == 0), stop=(hf == 1))
                sq = p1.tile([128, 512], F32, tag="sqt")
                nc.scalar.activation(sq[:], qT[hf][:, sl], ACTF.Square)
                nc.tensor.matmul(ps2[:], ones_f[:], sq[:],
                                 start=(hf == 0), stop=(hf == 1))
            nc.vector.tensor_copy(srow[:, sl], ps[:])
            nc.vector.tensor_copy(s2row[:, sl], ps2[:])

        mean = p1.tile([1, NQT], F32, tag="l1m")
        nc.vector.tensor_scalar(mean[:], srow[:], 1.0 / C, None, alu.mult)
        var = p1.tile([1, NQT], F32, tag="l1v")
        nc.vector.tensor_scalar(var[:], s2row[:], 1.0 / C, None, alu.mult)
        msq = p1.tile([1, NQT], F32, tag="l1mq")
        nc.vector.tensor_tensor(msq[:], mean[:], mean[:], alu.mult)
        nc.vector.tensor_tensor(var[:], var[:], msq[:], alu.subtract)
        rs = p1.tile([1, NQT], F32, tag="l1r")
        sqv = p1.tile([1, NQT], F32, tag="l1sq", name="l1sqv")
        nc.scalar.activation(sqv[:], var[:], ACTF.Sqrt, bias=1e-5)
        nc.vector.reciprocal(rs[:], sqv[:])
        mrs = p1.tile([1, NQT], F32, tag="l1mr")
        nc.vector.tensor_tensor(mrs[:], mean[:], rs[:], alu.mult)
        RS = bcast_row(rs[:], NQT, "RSb", p1)
        MRS = bcast_row(mrs[:], NQT, "MRSb", p1)

        for hf in range(2):
            nc.sync.dma_start(qT_d[:, hf * NQT:(hf + 1) * NQT], qT[hf][:])
            qn = p1.tile([128, NQT], F32, tag="qn")
            nc.vector.tensor_tensor(qn[:], qT[hf][:], RS[:], alu.mult)
            nc.vector.tensor_tensor(qn[:], qn[:], MRS[:], alu.subtract)
            nc.vector.tensor_scalar(qn[:], qn[:], g1_c[hf][:], b1_c[hf][:],
                                    alu.mult, alu.add)
            nc.sync.dma_start(qnT_d[:, hf * NQT:(hf + 1) * NQT], qn[:])
            qa32 = p1.tile([128, NQT], F32, tag="qa32")
            nc.vector.tensor_tensor(qa32[:], qn[:], pT[hf][:], alu.add)
            nc.scalar.activation(qaT[hf][:], qa32[:], ACTF.Copy)

    # ---- phase 2: value tables ----
    tables = [perm.tile([128, NV], F32, tag=f"tab{s}") for s in range(B)]
    with tc.tile_pool(name="vp", bufs=2) as vp:
        for b in range(B):
            vT = [vp.tile([128, NV], BF16, tag=f"vT{hf}") for hf in range(2)]
            for vt in range((NV + 127) // 128):
                r0 = vt * 128
                rn = min(128, NV - r0)
                l32 = vp.tile([128, C], F32, tag="l32")
                lv = vp.tile([128, C], BF16, tag="lv")
                nc.sync.dma_start(l32[:rn, :], dr["value"][b, r0:r0 + rn, :])
                nc.scalar.activation(lv[:rn, :], l32[:rn, :], ACTF.Copy)
                for hf in range(2):
                    ps = psp.tile([128, 128], BF16, tag="tp")
                    nc.tensor.transpose(ps[:, :rn], lv[:rn, hf * 128:(hf + 1) * 128],
                                        ident_b[:rn, :rn])
                    nc.vector.tensor_copy(vT[hf][:, r0:r0 + rn], ps[:, :rn])
            for par in range(2):
                for chu in range((NV + 511) // 512):
                    c0 = chu * 512
                    cn = min(512, NV - c0)
                    ps = psp.tile([128, 512], F32, tag="ps1")
                    for hf in range(2):
                        WvM = Wv_b[hf][:].rearrange(
                            "k (hc two) -> k hc two", two=2)[:, :, par:par + 1]
                        nc.tensor.matmul(ps[:, :cn], WvM, vT[hf][:, c0:c0 + cn],
                                         start=(hf == 0), stop=(hf == 1))
                    dst = tables[b][:, c0:c0 + cn].bitcast(BF16).rearrange(
                        "p (n two) -> p n two", two=2)[:, :, par:par + 1]
                    nc.scalar.activation(dst, ps[:, :cn], ACTF.Identity,
                                         bias=bv_c[par][:])

    # ---- phases 3+4 (per b): offsets, aw, coords, streams ----
    arrs = [perm.tile([128, NJ * NQS // 16], I16, tag=f"arr{s}") for s in range(B)]
    wdup_d = dramp.tile([NHLP, 4 * B * NQS * 2], BF16, tag="wdup_d")

    with tc.tile_pool(name="cp", bufs=1) as cp, \
         tc.tile_pool(name="ct", bufs=2) as ct:
        # awT for both b at once
        awT = cp.tile([NHLP, NQT], F32, tag="awT")
        for t in range(16):
            sl = slice(t * 128, (t + 1) * 128)
            ps = psp.tile([128, 96], F32, tag="ps1")
            for hf in range(2):
                nc.tensor.matmul(ps[:], qaT[hf][:, sl],
                                 Wa_b[hf][:], start=(hf == 0), stop=(hf == 1))
            z = ct.tile([128, 96], F32, tag="z")
            nc.vector.tensor_tensor(z[:], ps[:], baT[:], alu.add)
            zg = z[:].rearrange("p (h lp) -> p h lp", h=H)
            mx = ct.tile([128, H], F32, tag="mx")
            nc.vector.tensor_reduce(mx[:], zg, AX.X, alu.max)
            nc.vector.tensor_tensor(
                zg, zg, mx[:].unsqueeze(2).broadcast_to([128, H, NLP]),
                alu.subtract)
            ez = ct.tile([128, 96], F32, tag="ez")
            nc.scalar.activation(ez[:], z[:], ACTF.Exp)
            sm = ct.tile([128, H], F32, tag="sm")
            nc.vector.tensor_reduce(sm[:], ez[:].rearrange("p (h lp) -> p h lp", h=H),
                                    AX.X, alu.add)
            rc = ct.tile([128, H], F32, tag="rc")
            nc.vector.reciprocal(rc[:], sm[:])
            awq = ct.tile([128, 96], F32, tag="awq")
            nc.vector.tensor_tensor(
                awq[:].rearrange("p (h lp) -> p h lp", h=H),
                ez[:].rearrange("p (h lp) -> p h lp", h=H),
                rc[:].unsqueeze(2).broadcast_to([128, H, NLP]),
                alu.mult)
            ps2 = psp.tile([96, 128], F32, tag="tp")
            nc.tensor.transpose(ps2[:], awq[:], ident_f[:])
            nc.vector.tensor_copy(awT[:, sl], ps2[:])

        # refT [6, NQT]
        refT = cp.tile([6, NQT], F32, tag="refT")
        for t in range(16):
            tl = ct.tile([128, 6], F32, tag="refl")
            nc.sync.dma_start(tl[:], dr["ref_pts"][t * 128:(t + 1) * 128, :])
            ps = psp.tile([6, 128], F32, tag="tp")
            nc.tensor.transpose(ps[:], tl[:], ident_f[:])
            nc.vector.tensor_copy(refT[:, t * 128:(t + 1) * 128], ps[:])

        for b in range(B):
            vsl = slice(b * NQS, (b + 1) * NQS)
            cres = {}
            for xy in range(2):
                nrm, m1, m2 = ((col(0), col(1), col(2)) if xy == 0 else
                               (col(3), col(4), col(5)))
                # gxs = offs + (ref*nrm + 1023.5)
                gxs = ct.tile([NHLP, NQS], F32, tag="gxs")
                for chu in range(NQS // 512):
                    sl = slice(chu * 512, (chu + 1) * 512)
                    gsl = slice(b * NQS + chu * 512, b * NQS + (chu + 1) * 512)
                    ps = psp.tile([NHLP, 512], F32, tag="ps1")
                    for hf in range(2):
                        WoM = Wo_b[hf][:].rearrange(
                            "k (r two) -> k r two", two=2)[:, :, xy:xy + 1]
                        nc.tensor.matmul(ps[:], WoM, qaT[hf][:, gsl],
                                         start=(hf == 0), stop=(hf == 1))
                    nc.scalar.activation(gxs[:, sl], ps[:], ACTF.Identity,
                                         bias=bo_c[xy][:])
                rrep = ct.tile([NHLP, NQS], F32, tag="rrep")
                for l in range(L):
                    src = refT[l * 2 + xy:l * 2 + xy + 1, vsl]
                    for h in range(H):
                        r0 = h * NLP + l * P
                        for p in range(P):
                            nc.sync.dma_start(rrep[r0 + p:r0 + p + 1, :], src)
                rsc = ct.tile([NHLP, NQS], F32, tag="rsc")
                nc.scalar.activation(rsc[:], rrep[:], ACTF.Copy, bias=1023.5,
                                     scale=nrm)
                nc.vector.tensor_tensor(gxs[:], gxs[:], rsc[:], alu.add)
                # floor (robust to trunc or round casts)
                x0i = ct.tile([NHLP, NQS], I32, tag="x0i")
                nc.vector.tensor_copy(x0i[:], gxs[:])
                x0s = ct.tile([NHLP, NQS], F32, tag="x0s")
                nc.vector.tensor_copy(x0s[:], x0i[:])
                fx = ct.tile([NHLP, NQS], F32, tag="fx")
                nc.vector.tensor_tensor(fx[:], gxs[:], x0s[:], alu.subtract)
                neg = ct.tile([NHLP, NQS], F32, tag="neg")
                nc.vector.tensor_scalar(neg[:], fx[:], 0.0, None, alu.is_lt)
                nc.vector.tensor_tensor(x0s[:], x0s[:], neg[:], alu.subtract)
                nc.vector.tensor_tensor(fx[:], fx[:], neg[:], alu.add)
                x0 = ct.tile([NHLP, NQS], F32, tag="x0")
                nc.vector.tensor_scalar(x0[:], x0s[:], -1024.0, None, alu.add)
                # masks and weights
                m0t = ct.tile([NHLP, NQS], F32, tag="m0")
                t2 = ct.tile([NHLP, NQS], F32, tag="t2")
                nc.vector.tensor_scalar(m0t[:], x0[:], 0.0, None, alu.is_ge)
                nc.vector.tensor_scalar(t2[:], x0[:], m1, None, alu.is_le)
                nc.vector.tensor_tensor(m0t[:], m0t[:], t2[:], alu.mult)
                m1t = ct.tile([NHLP, NQS], F32, tag="m1")
                nc.vector.tensor_scalar(m1t[:], x0[:], -1.0, None, alu.is_ge)
                nc.vector.tensor_scalar(t2[:], x0[:], m2, None, alu.is_le)
                nc.vector.tensor_tensor(m1t[:], m1t[:], t2[:], alu.mult)
                w0 = cp.tile([NHLP, NQS], F32, tag=f"w0_{xy}")
                nc.vector.tensor_scalar(w0[:], fx[:], -1.0, 1.0, alu.mult, alu.add)
                nc.vector.tensor_tensor(w0[:], w0[:], m0t[:], alu.mult)
                w1 = cp.tile([NHLP, NQS], F32, tag=f"w1_{xy}")
                nc.vector.tensor_tensor(w1[:], fx[:], m1t[:], alu.mult)
                xc0 = cp.tile([NHLP, NQS], F32, tag=f"xc0_{xy}")
                nc.vector.tensor_scalar(xc0[:], x0[:], 0.0, m1, alu.max, alu.min)
                xc1 = cp.tile([NHLP, NQS], F32, tag=f"xc1_{xy}")
                nc.vector.tensor_scalar(xc1[:], x0[:], 1.0, 0.0, alu.add, alu.max)
                nc.vector.tensor_scalar(xc1[:], xc1[:], m1, None, alu.min)
                if xy == 0:
                    cres["xc"] = (xc0, xc1); cres["wx"] = (w0, w1)
                else:
                    nc.vector.tensor_scalar(xc0[:], xc0[:], col(6), col(7),
                                            alu.mult, alu.add)
                    nc.vector.tensor_scalar(xc1[:], xc1[:], col(6), col(7),
                                            alu.mult, alu.add)
                    cres["yb"] = (xc0, xc1); cres["wy"] = (w0, w1)

            for blk in range(4):
                row, x = blk // 2, blk % 2
                pxb = ct.tile([NHLP, NQS], F32, tag="pxb")
                nc.vector.tensor_tensor(pxb[:], cres["yb"][row][:],
                                        cres["xc"][x][:], alu.add)
                pxi = ct.tile([NHLP, NQS], I16, tag="pxi")
                nc.vector.tensor_copy(pxi[:], pxb[:])
                wb = ct.tile([NHLP, NQS], F32, tag="wb")
                nc.vector.tensor_tensor(wb[:], cres["wy"][row][:],
                                        cres["wx"][x][:], alu.mult)
                nc.vector.tensor_tensor(wb[:], wb[:], awT[:, vsl], alu.mult)
                wdup = ct.tile([NHLP, NQS * 2], BF16, tag="wdup")
                nc.vector.tensor_copy(
                    wdup[:].rearrange("p (n two) -> p n two", two=2),
                    wb[:].unsqueeze(2).broadcast_to([NHLP, NQS, 2]))
                for lp in range(NLP):
                    j = blk * NLP + lp
                    for h in range(H):
                        nc.sync.dma_start(
                            arrs[b][16 * h:16 * (h + 1), j * 64:(j + 1) * 64],
                            pxi[h * NLP + lp:h * NLP + lp + 1, :])
                base = (blk * B + b) * NQS * 2
                nc.sync.dma_start(wdup_d[:, base:base + NQS * 2], wdup[:])

    # ---- phase 5: gather + combine ----
    sampled = [perm.tile([128, NQS], F32, tag=f"smp{s}") for s in range(B)]
    with tc.tile_pool(name="gp", bufs=2) as gp, \
         tc.tile_pool(name="wpp", bufs=2) as wpp:
        for s in range(B):
            for ch in range(NCHUNK):
                G = gp.tile([128, CHL], F32, tag="G")
                nc.gpsimd.ap_gather(G[:], tables[s][:],
                                    arrs[s][:, ch * 192:(ch + 1) * 192],
                                    channels=128, num_elems=NV, d=1, num_idxs=CHL)
                Wsrc = wpp.tile([128, CHL], F32, tag="Wsrc")
                for jj in range(JC):
                    j = ch * JC + jj
                    blk, lp = j // NLP, j % NLP
                    base = (blk * B + s) * NQS * 2
                    for h in range(H):
                        nc.sync.dma_start(
                            Wsrc[16 * h:16 * h + 1,
                                 jj * NQS:(jj + 1) * NQS].bitcast(BF16),
                            wdup_d[h * NLP + lp:h * NLP + lp + 1,
                                   base:base + NQS * 2])
                Wb = wpp.tile([128, CHL], F32, tag="Wb")
                nc.vector.stream_shuffle(
                    Wb[:].rearrange("p (j m r) -> p j m r", j=JC, m=64, r=16),
                    Wsrc[:].rearrange("p (j r m) -> p j m r", j=JC, r=16, m=64),
                    [0] * 16 + [16] * 16)
                gb = G[:].bitcast(BF16)
                nc.vector.tensor_tensor(gb, gb, Wb[:].bitcast(BF16), alu.mult)
                nq2 = NQS * 2
                nc.vector.tensor_tensor(gb[:, 0:nq2], gb[:, 0:nq2],
                                        gb[:, nq2:2 * nq2], alu.add)
                nc.vector.tensor_tensor(gb[:, 0:nq2], gb[:, 0:nq2],
                                        gb[:, 2 * nq2:3 * nq2], alu.add)
                if ch == 0:
                    nc.vector.tensor_copy(sampled[s][:].bitcast(BF16), gb[:, 0:nq2])
                else:
                    nc.vector.tensor_tensor(sampled[s][:].bitcast(BF16),
                                            sampled[s][:].bitcast(BF16),
                                            gb[:, 0:nq2], alu.add)

    # ---- phase 6: Wp proj + residuals + LN2 + FFN + store ----
    with tc.tile_pool(name="f6", bufs=1) as f6, \
         tc.tile_pool(name="fs", bufs=2) as fs:
        qrT = [f6.tile([128, NQT], F32, tag=f"qrT{i}") for i in range(2)]
        for b in range(B):
            rhs_par = [
                sampled[b][:].bitcast(BF16).rearrange(
                    "p (m r two) -> p r m two", m=64, r=16, two=2)[:, :, :, par:par + 1]
                for par in range(2)]
            for mh in range(2):
                for vc in range(NQS // 512):
                    ps = psp.tile([128, 512], F32, tag="ps1")
                    for par in range(2):
                        rhs_c = rhs_par[par][:, vc * 8:(vc + 1) * 8, :, :].squeeze(3)
                        nc.tensor.matmul(ps[:],
                                         Wp_par[par][:, mh * 128:(mh + 1) * 128],
                                         rhs_c, start=(par == 0), stop=(par == 1))
                    gsl = slice(b * NQS + vc * 512, b * NQS + (vc + 1) * 512)
                    at = fs.tile([128, 512], F32, tag="at")
                    nc.scalar.activation(at[:], ps[:], ACTF.Identity, bias=bp_c[mh][:])
                    qn_c = fs.tile([128, 512], F32, tag="qn_c")
                    nc.sync.dma_start(qn_c[:], qnT_d[:, mh * NQT + b * NQS + vc * 512:
                                                     mh * NQT + b * NQS + (vc + 1) * 512])
                    qt_c = fs.tile([128, 512], F32, tag="qt_c")
                    nc.sync.dma_start(qt_c[:], qT_d[:, mh * NQT + b * NQS + vc * 512:
                                                    mh * NQT + b * NQS + (vc + 1) * 512])
                    nc.vector.tensor_tensor(at[:], at[:], qn_c[:], alu.add)
                    nc.vector.tensor_tensor(qrT[mh][:, gsl], at[:], qt_c[:], alu.add)

        # LN2 stats
        srow2 = f6.tile([1, NQT], F32, tag="l2s")
        s2row2 = f6.tile([1, NQT], F32, tag="l2s2")
        for chu in range(NQT // 512):
            sl = slice(chu * 512, (chu + 1) * 512)
            ps = psp.tile([1, 512], F32, tag="ps1")
            ps2 = psp.tile([1, 512], F32, tag="ps2")
            for hf in range(2):
                nc.tensor.matmul(ps[:], ones_f[:], qrT[hf][:, sl],
                                 start=(hf == 0), stop=(hf == 1))
                sq = fs.tile([128, 512], F32, tag="sq2")
                nc.scalar.activation(sq[:], qrT[hf][:, sl], ACTF.Square)
                nc.tensor.matmul(ps2[:], ones_f[:], sq[:],
                                 start=(hf == 0), stop=(hf == 1))
            nc.vector.tensor_copy(srow2[:, sl], ps[:])
            nc.vector.tensor_copy(s2row2[:, sl], ps2[:])
        mean2 = f6.tile([1, NQT], F32, tag="l2m")
        nc.vector.tensor_scalar(mean2[:], srow2[:], 1.0 / C, None, alu.mult)
        var2 = f6.tile([1, NQT], F32, tag="l2v")
        nc.vector.tensor_scalar(var2[:], s2row2[:], 1.0 / C, None, alu.mult)
        msq2 = f6.tile([1, NQT], F32, tag="l2mq")
        nc.vector.tensor_tensor(msq2[:], mean2[:], mean2[:], alu.mult)
        nc.vector.tensor_tensor(var2[:], var2[:], msq2[:], alu.subtract)
        rs2 = f6.tile([1, NQT], F32, tag="l2r")
        sqv2 = f6.tile([1, NQT], F32, tag="l2sq", name="l2sqv")
        nc.scalar.activation(sqv2[:], var2[:], ACTF.Sqrt, bias=1e-5)
        nc.vector.reciprocal(rs2[:], sqv2[:])
        mrs2 = f6.tile([1, NQT], F32, tag="l2mr")
        nc.vector.tensor_tensor(mrs2[:], mean2[:], rs2[:], alu.mult)
        RS2 = bcast_row(rs2[:], NQT, "RS2b", f6)
        MRS2 = bcast_row(mrs2[:], NQT, "MRS2b", f6)

        # FFN per 512-chunk
        for vc in range(NQT // 512):
            sl = slice(vc * 512, (vc + 1) * 512)
            q2c = []
            for hf in range(2):
                t = fs.tile([128, 512], F32, tag="q2w")
                nc.vector.tensor_tensor(t[:], qrT[hf][:, sl], RS2[:, sl], alu.mult)
                nc.vector.tensor_tensor(t[:], t[:], MRS2[:, sl], alu.subtract)
                nc.vector.tensor_scalar(t[:], t[:], g2_c[hf][:], b2_c[hf][:],
                                        alu.mult, alu.add)
                tb = fs.tile([128, 512], BF16, tag=f"q2b{hf}")
                nc.scalar.activation(tb[:], t[:], ACTF.Copy)
                q2c.append(tb)
            gel = []
            for mt in range(8):
                ps = psp.tile([128, 512], F32, tag="ps1")
                for hf in range(2):
                    nc.tensor.matmul(ps[:],
                                     Wf1_b[hf][:, mt * 128:(mt + 1) * 128],
                                     q2c[hf][:], start=(hf == 0), stop=(hf == 1))
                gl = fs.tile([128, 512], BF16, tag=f"gel{mt}")
                nc.scalar.activation(gl[:], ps[:], ACTF.Gelu, bias=bf1_c[mt][:])
                gel.append(gl)
            for mh in range(2):
                ps = psp.tile([128, 512], F32, tag="ps1")
                for kt in range(8):
                    nc.tensor.matmul(ps[:],
                                     Wf2_b[kt][:, mh * 128:(mh + 1) * 128],
                                     gel[kt][:], start=(kt == 0), stop=(kt == 7))
                ff = fs.tile([128, 512], F32, tag="ff")
                nc.scalar.activation(ff[:], ps[:], ACTF.Identity, bias=bf2_c[mh][:])
                nc.vector.tensor_tensor(ff[:], ff[:], qrT[mh][:, sl], alu.add)
                for qt in range(4):
                    ps2 = psp.tile([128, 128], F32, tag="tp")
                    nc.tensor.transpose(ps2[:], ff[:, qt * 128:(qt + 1) * 128],
                                        ident_f[:])
                    ot = fs.tile([128, 128], F32, tag="ot")
                    nc.vector.tensor_copy(ot[:], ps2[:])
                    q0 = vc * 512 + qt * 128
                    nc.sync.dma_start(
                        dr["out"][q0:q0 + 128, mh * 128:(mh + 1) * 128], ot[:])


# ======================== host driver ========================
_CACHE = {}


def _get_compiled():
    if "nc" not in _CACHE:
        import concourse.bacc as bacc
        nc = bacc.Bacc("TRN2", target_bir_lowering=False, debug=False,
                       enable_asserts=False, num_devices=8)
        build(nc)
        nc.compile()
        _CACHE["nc"] = nc
    return _CACHE["nc"]


def _in_maps(inputs):
    consts = host_consts()
    full = {k: np.ascontiguousarray(np.asarray(v, np.float32))
            for k, v in inputs.items()
            if k not in ("spatial_shapes", "level_start_index")}
    maps = []
    for k in range(8):
        qsl = slice(k * NQS, (k + 1) * NQS)
        m = {
            "query": full["query"][:, qsl, :].reshape(NQT, C),
            "value": full["value"],
            "query_pos": full["query_pos"][:, qsl, :].reshape(NQT, C),
            "ref_pts": full["ref_pts"][:, qsl, :, :].reshape(NQT, 6),
            "Wo": full["Wo"], "Wa": full["Wa"], "Wv": full["Wv"],
            "Wp": full["Wp"], "Wf1": full["Wf1"], "Wf2": full["Wf2"],
            "g1": full["g1"].reshape(1, -1), "b1": full["b1"].reshape(1, -1),
            "g2": full["g2"].reshape(1, -1), "b2": full["b2"].reshape(1, -1),
            "bo": full["bo"].reshape(1, -1), "ba": full["ba"].reshape(1, -1),
            "bv": full["bv"].reshape(1, -1), "bp": full["bp"].reshape(1, -1),
            "bf1": full["bf1"].reshape(1, -1), "bf2": full["bf2"].reshape(1, -1),
            "ident": consts["ident"], "ccols": consts["ccols"],
        }
        maps.append({k2: np.ascontiguousarray(v) for k2, v in m.items()})
    return maps


def kernel(**inputs):
    from concourse import bass_utils
    nc = _get_compiled()
    maps = _in_maps(inputs)
    res = bass_utils.run_bass_kernel_spmd(nc, maps, core_ids=list(range(8)))
    Nq = 8 * NQS
    out = np.zeros((B, Nq, C), np.float32)
    for k in range(8):
        o = res.results[k]["out"].reshape(B, NQS, C)
        out[:, k * NQS:(k + 1) * NQS, :] = o
    return out
